# revision 59
# baseline (speedup 1.0000x reference)
"""Trainium2 Bass kernel for capsule dynamic routing (nn_Capsule).

Math (per sample):
  hat[i,(n,d)] = sum_d' x[i,d'] W[d',(n,d)]        (i=1024, d'=128, n=32, d=16)
  3 routing iters: c = softmax(b, axis=n); o = squash(sum_i c[n,i] hat[i,n,:])
                   b = sum_d o[n,d] hat[i,n,d]
Never materialize hat.  W columns permuted k' = d*32 + n so masked reduces are
contiguous and the mask is one [128,128] tile for every chunk.

Per group of 4 samples (stacked 4*32 = 128 partitions q=(b,n)) and iteration,
the work is split into 6 stages and EMITTED SOFTWARE-PIPELINED with a 1-stage
skew between groups, so every engine queue interleaves different stages of
different groups and nothing hard-barriers:
  S0: GT[d',q] += xn-chunk^T-stationary MMs;  GTs copy
  S1: F (512-col MM) + FT chunks;  ts4/s4/sq/ss (DVE);  tsTu = FT*maskT
  S2: newton-rsqrt scale [128,1];  flip to [1,q] (identity MM);  scB (K=1 MM)
  S3: tsTs = tsTu*scB;  HT += wtp^T MMs;  HTs copy
  S4: bt = xT-chunk MMs (i-part);  exp
  S5: z; rz; ct = e*rz (split DVE / GpSimd halves)
Final iter: S0, S1, then o = s*scale -> DMA out.
Sharding: data-parallel over batch, 16 samples/core x 8 cores.
"""

import os
import sys

sys.path.insert(0, "/opt/trn_rl_repo")

import numpy as np

import concourse.bass as bass
import concourse.bacc as bacc
import concourse.mybir as mybir
from concourse import tile
from concourse.bass_utils import run_bass_kernel_spmd

FP32 = mybir.dt.float32
BF16 = mybir.dt.bfloat16
FP8 = mybir.dt.float8e4
I32 = mybir.dt.int32
AF = mybir.ActivationFunctionType
AX = mybir.AxisListType
AL = mybir.AluOpType

EPS = 1e-7
N_CORES = 8
B_TOTAL, IN, D = 128, 1024, 128
NCAP, DC = 32, 16
K = NCAP * DC
B_LOC = B_TOTAL // N_CORES
GSZ = 4
NG = B_LOC // GSZ
NCH = IN // 128


def build():
    nc = bacc.Bacc("TRN2", target_bir_lowering=False)
    xT = nc.declare_dram_parameter("xT", [B_LOC, D, IN], BF16, isOutput=False)
    xn = nc.declare_dram_parameter("xn", [B_LOC, 128, NCH, D], BF16, isOutput=False)
    wp = nc.declare_dram_parameter("wp", [D, K], BF16, isOutput=False)
    wpc = nc.declare_dram_parameter("wpc", [D, 4, 128], BF16, isOutput=False)
    wtp = nc.declare_dram_parameter("wtp", [K, D], BF16, isOutput=False)
    maskp = nc.declare_dram_parameter("maskp", [128, K], BF16, isOutput=False)
    maskt = nc.declare_dram_parameter("maskt", [128, 128], BF16, isOutput=False)
    ident = nc.declare_dram_parameter("ident", [128, 128], BF16, isOutput=False)
    out = nc.declare_dram_parameter("out", [B_LOC, NCAP, DC], FP32, isOutput=True)

    with tile.TileContext(nc) as tc:
        with (
            tc.tile_pool(name="const", bufs=1) as cpool,
            tc.tile_pool(name="xp", bufs=1) as xp,
            tc.tile_pool(name="sbp", bufs=6) as sbp,
            tc.tile_pool(name="tsp", bufs=6) as tsp,
            tc.tile_pool(name="ep", bufs=6) as ep,
            tc.tile_pool(name="ctp", bufs=8) as ctp,
            tc.tile_pool(name="small", bufs=16) as smallp,
            tc.tile_pool(name="gt", bufs=2, space="PSUM") as gtp,
            tc.tile_pool(name="fn", bufs=1, space="PSUM") as fnp,
            tc.tile_pool(name="ft", bufs=1, space="PSUM") as ftp,
            tc.tile_pool(name="sc", bufs=1, space="PSUM") as scp,
            tc.tile_pool(name="ht", bufs=1, space="PSUM") as htp,
            tc.tile_pool(name="bt", bufs=1, space="PSUM") as btp,
        ):
            # xn group 0 first so the pipeline ramps immediately
            xn_t = []
            for g in range(NG):
                t2 = xp.tile([128, GSZ, NCH, D], BF16, tag=f"xn{g}",
                             name=f"xng{g}")
                xn_t.append(t2)
            nc.sync.dma_start(xn_t[0][:, 0], xn[0])
            nc.sync.dma_start(
                xn_t[0][:, 1:GSZ],
                xn[1:GSZ].rearrange("b p c d -> p b c d"))
            wp_sb = cpool.tile([D, K], BF16, tag="wp")
            nc.sync.dma_start(wp_sb[:], wp[:])
            wpc_sb = cpool.tile([D, 4, 128], BF16, tag="wpc")
            nc.sync.dma_start(wpc_sb[:], wpc[:])
            wtp_sb = cpool.tile([128, 4, D], BF16, tag="wtp")
            nc.sync.dma_start(wtp_sb[:], wtp.rearrange("(j p) d -> p j d", p=128))
            mp_sb = cpool.tile([128, K], BF16, tag="maskp")
            nc.sync.dma_start(mp_sb[:], maskp[:])
            mt_sb = cpool.tile([128, 128], BF16, tag="maskt")
            nc.sync.dma_start(mt_sb[:], maskt[:])
            id_sb = cpool.tile([128, 128], BF16, tag="ident")
            nc.sync.dma_start(id_sb[:], ident[:])
            c0_sb = cpool.tile([128, NCAP], BF16, tag="c0")
            nc.vector.memset(c0_sb[:], 1.0 / NCAP)
            ones128 = cpool.tile([128, 128], BF16, tag="ones128")
            nc.vector.memset(ones128[:], 1.0)

            xn_g = xn_t
            xT_g = []
            for g in range(NG):
                t = xp.tile([128, GSZ, IN], BF16, tag=f"xT{g}", name=f"xTg{g}")
                xT_g.append(t)
            # remaining xn (group 0 already queued above), then all xT
            for g in range(1, NG):
                nc.sync.dma_start(
                    xn_g[g][:],
                    xn[g * GSZ:(g + 1) * GSZ].rearrange("b p c d -> p b c d"))
            for g in range(NG):
                nc.sync.dma_start(
                    xT_g[g][:],
                    xT[g * GSZ:(g + 1) * GSZ].rearrange("b p i -> p b i"))

            # persistent cross-stage state, per group
            st = [dict() for _ in range(NG)]
            ct = [None] * NG

            def s0(g, it):
                GT4 = gtp.tile([128, 128], FP32, tag="gt4")
                for b in range(GSZ):
                    for c in range(NCH):
                        mv = c0_sb[:] if it == 0 else ct[g][:, b, c, :]
                        nc.tensor.matmul(
                            GT4[:, 32 * b:32 * b + 32],
                            xn_g[g][:, b, c, :],
                            mv,
                            start=(c == 0),
                            stop=(c == NCH - 1),
                        )
                Gs = sbp.tile([128, 128], BF16, tag="gts")
                nc.scalar.copy(Gs[:], GT4[:])
                st[g]["GTs"] = Gs

            def s1(g, it):
                Gs = st[g].pop("GTs")
                F4t = fnp.tile([128, 4, 128], FP32, tag="f4n", name="F4n")
                F4 = F4t[:].rearrange("p j q -> p (j q)")
                ss4 = smallp.tile([128, 1], FP32, tag="ss4")
                if it < 2:
                    # FT first: feeds the DVE queue's first op (tsTu) earlier
                    FT4 = ftp.tile([128, 4, 128], FP32, tag="ft4t", name="FT4")
                    for j in range(4):
                        nc.tensor.matmul(
                            FT4[:, j, :], wpc_sb[:, j, :], Gs[:],
                            start=True, stop=True,
                        )
                    nc.tensor.matmul(F4, Gs[:], wp_sb[:], start=True, stop=True)
                    tsTu = tsp.tile([128, 4, 128], BF16, tag="tstu")
                    nc.vector.tensor_mul(
                        tsTu[:], FT4[:],
                        mt_sb[:].rearrange("p (a q) -> p a q", a=1)
                        .to_broadcast([128, 4, 128]),
                    )
                    st[g]["tsTu"] = tsTu
                    ts4 = tsp.tile([128, K], BF16, tag="ts4")
                    nc.vector.tensor_mul(ts4[:], F4, mp_sb[:])
                    # ss = sum((F*mask)^2): masked rows hold s_d exactly once
                    dead = tsp.tile([128, K], BF16, tag="dead")
                    nc.scalar.activation(dead[:], ts4[:], AF.Square,
                                         accum_out=ss4[:])
                else:
                    nc.tensor.matmul(F4, Gs[:], wp_sb[:], start=True, stop=True)
                    ts4 = tsp.tile([128, K], BF16, tag="ts4")
                    nc.vector.tensor_mul(ts4[:], F4, mp_sb[:])
                    s4 = smallp.tile([128, DC], FP32, tag="s4")
                    nc.vector.reduce_sum(
                        s4[:], ts4[:].rearrange("p (d n) -> p d n", d=DC),
                        axis=AX.X,
                    )
                    dead = tsp.tile([128, K], BF16, tag="dead")
                    nc.scalar.activation(dead[:], ts4[:], AF.Square,
                                         accum_out=ss4[:])
                    st[g]["s4"] = s4
                st[g]["ss4"] = ss4

            def s2(g, it):
                # newton-rsqrt scale, per-partition [128,1]
                ss4 = st[g].pop("ss4")
                p = smallp
                ve = p.tile([128, 1], FP32, tag="ve")
                nc.vector.tensor_scalar_add(ve[:], ss4[:], EPS)
                ib = p.tile([128, 1], I32, tag="ib")
                nc.vector.tensor_scalar(ib[:], ve[:].bitcast(I32), 1, None,
                                        op0=AL.arith_shift_right)
                nc.vector.tensor_scalar(ib[:], ib[:], -1, 0x5F3759DF,
                                        op0=AL.mult, op1=AL.add)
                y0 = ib[:].bitcast(FP32)
                aN = p.tile([128, 1], FP32, tag="aN")
                yN = p.tile([128, 1], FP32, tag="yN")
                nc.vector.tensor_mul(aN[:], y0, y0)
                nc.vector.tensor_mul(aN[:], aN[:], ve[:])
                nc.vector.tensor_scalar(aN[:], aN[:], -0.5, 1.5,
                                        op0=AL.mult, op1=AL.add)
                nc.vector.tensor_mul(yN[:], y0, aN[:])
                sv = p.tile([128, 1], FP32, tag="sv")
                nc.vector.tensor_mul(sv[:], yN[:], ve[:])
                den = p.tile([128, 1], FP32, tag="den")
                nc.vector.tensor_scalar_add(den[:], ve[:], 0.5)
                rden = p.tile([128, 1], FP32, tag="rden")
                nc.vector.reciprocal(rden[:], den[:])
                if it == 2:
                    sc4 = p.tile([128, 1], FP32, tag="sc4")
                    nc.vector.tensor_mul(sc4[:], sv[:], rden[:])
                    o4 = p.tile([128, DC], FP32, tag="o4")
                    nc.vector.tensor_scalar_mul(o4[:], st[g].pop("s4")[:],
                                                sc4[:])
                    nc.sync.dma_start(
                        out[g * GSZ:(g + 1) * GSZ].rearrange("b n d -> (b n) d"),
                        o4[:],
                    )
                    return
                sc4 = p.tile([128, 1], FP32, tag="sc4")
                nc.vector.tensor_mul(sc4[:], sv[:], rden[:])
                # broadcast scale along free on ACT, then one PE transpose
                # puts it in [p, q]-orientation for the tsTs column scaling
                scBT = sbp.tile([128, 128], BF16, tag="scbt")
                nc.scalar.mul(scBT[:], ones128[:], sc4[:])
                scB = scp.tile([128, 256], BF16, tag="scb", name="scBps")
                nc.tensor.transpose(scB[:, 0:128], scBT[:], id_sb[:])
                scBs = sbp.tile([128, 128], BF16, tag="scbs")
                nc.scalar.copy(scBs[:], scB[:, 0:128])
                st[g]["scBs"] = scBs

            def s3(g, it):
                tsTu = st[g].pop("tsTu")
                scBs = st[g].pop("scBs")
                tsTs = tsp.tile([128, 4, 128], BF16, tag="tsts")
                nc.vector.tensor_mul(
                    tsTs[:], tsTu[:],
                    scBs[:].rearrange("p (a q) -> p a q", a=1)
                    .to_broadcast([128, 4, 128]),
                )
                HTu = htp.tile([128, 128], FP32, tag="htu")
                for j in range(4):
                    nc.tensor.matmul(
                        HTu[:], wtp_sb[:, j, :], tsTs[:, j, :],
                        start=(j == 0), stop=(j == 3),
                    )
                HTs = sbp.tile([128, 128], BF16, tag="hts")
                nc.scalar.copy(HTs[:], HTu[:])
                st[g]["HTs"] = HTs

            def s4stage(g, it):
                HTs = st[g].pop("HTs")
                e4 = ep.tile([128, GSZ, NCH, NCAP], BF16, tag="e4")
                bt4 = btp.tile([128, GSZ, NCH, NCAP], FP32, tag="bt4")
                for b in range(GSZ):
                    for c in range(NCH):
                        nc.tensor.matmul(
                            bt4[:, b, c, :],
                            xT_g[g][:, b, 128 * c:128 * c + 128],
                            HTs[:, 32 * b:32 * b + 32],
                            start=True,
                            stop=True,
                        )
                nc.scalar.activation(
                    e4[:].rearrange("p a c n -> p (a c n)"),
                    bt4[:].rearrange("p a c n -> p (a c n)"),
                    AF.Exp,
                )
                st[g]["e4"] = e4

            def s5(g, it):
                e4 = st[g].pop("e4")
                z4 = smallp.tile([128, GSZ * NCH], FP32, tag="z4")
                nc.vector.reduce_sum(z4[:], e4[:], axis=AX.X)
                rz4 = smallp.tile([128, GSZ * NCH], BF16, tag="rz4")
                with nc.allow_low_precision("softmax denominators O(1-30)"):
                    nc.vector.reciprocal(rz4[:], z4[:])
                ctg = ctp.tile([128, GSZ, NCH, NCAP], BF16, tag="ct4")
                rzv = rz4[:].rearrange("p (b c) -> p b c", b=GSZ)
                nc.vector.tensor_mul(
                    ctg[:, 0:2], e4[:, 0:2],
                    rzv[:, 0:2].to_broadcast([128, 2, NCH, NCAP]),
                )
                nc.gpsimd.tensor_mul(
                    ctg[:, 2:4], e4[:, 2:4],
                    rzv[:, 2:4].to_broadcast([128, 2, NCH, NCAP]),
                )
                ct[g] = ctg

            # stage list per group: 3 iterations, last one truncated
            STAGES = []
            for it in range(2):
                STAGES += [(s0, it), (s1, it), (s2, it), (s3, it),
                           (s4stage, it), (s5, it)]
            STAGES += [(s0, 2), (s1, 2), (s2, 2)]

            NS = len(STAGES)
            SKEW = 1
            for r in range(NS + SKEW * (NG - 1)):
                for g in range(NG):
                    s = r - SKEW * g
                    if 0 <= s < NS:
                        fn, it = STAGES[s]
                        fn(g, it)
    nc.compile()
    return nc


LAST_RESULT = None
_CONSTS = None


def _consts():
    global _CONSTS
    if _CONSTS is None:
        perm = np.empty(K, np.int64)
        for n in range(NCAP):
            for d in range(DC):
                perm[d * NCAP + n] = n * DC + d
        m32 = np.tile(np.eye(NCAP, dtype=np.float32), (1, DC)).reshape(NCAP, K)
        maskp = np.tile(m32, (GSZ, 1))
        pp, qq = np.meshgrid(np.arange(128), np.arange(128), indexing="ij")
        maskt = (pp % 32 == qq % 32).astype(np.float32)
        _CONSTS = (perm, maskp, maskt)
    return _CONSTS


def kernel(inputs, kernel):
    import ml_dtypes
    bf16 = ml_dtypes.bfloat16
    x = np.ascontiguousarray(np.asarray(inputs, dtype=np.float32))
    W = np.ascontiguousarray(np.asarray(kernel, dtype=np.float32)[0])
    xTh = np.ascontiguousarray(x.transpose(0, 2, 1).astype(bf16))
    xnL = np.ascontiguousarray(
        x.reshape(B_TOTAL, NCH, 128, D).transpose(0, 2, 1, 3).astype(bf16)
    )
    perm, maskp, maskt = _consts()
    WPf = W[:, perm]
    WP = np.ascontiguousarray(WPf.astype(bf16))
    WPC = np.ascontiguousarray(WPf.reshape(D, 4, 128).astype(bf16))
    WTP = np.ascontiguousarray(WPf.T.astype(bf16))

    nc = build()
    in_maps = [
        {
            "xT": xTh[i * B_LOC:(i + 1) * B_LOC],
            "xn": xnL[i * B_LOC:(i + 1) * B_LOC],
            "wp": WP,
            "wpc": WPC,
            "wtp": WTP,
            "maskp": maskp.astype(bf16),
            "maskt": maskt.astype(bf16),
            "ident": np.eye(128, dtype=np.float32).astype(bf16),
        }
        for i in range(N_CORES)
    ]
    res = run_bass_kernel_spmd(
        nc, in_maps, core_ids=list(range(N_CORES)),
        trace=bool(os.environ.get("KERNEL_TRACE")),
    )
    global LAST_RESULT
    LAST_RESULT = res
    return np.concatenate([res.results[i]["out"] for i in range(N_CORES)], axis=0)


if __name__ == "__main__":
    rng = np.random.default_rng(0)
    xi = rng.standard_normal((B_TOTAL, IN, D), dtype=np.float32)
    ki = (rng.standard_normal((1, D, K), dtype=np.float32) * 0.05).astype(np.float32)
    o = kernel(xi, ki)
    print(o.shape, o.dtype)
